# revision 2
# baseline (speedup 1.0000x reference)
"""Trainium2 Bass kernel for nn_Attention_41678362640976 (v2).

ViT-style attention with CLS-row prior injection. Data-parallel over batch:
one batch element per NeuronCore (B == 8 == n_cores).

Key design (cost-model-driven):
  - fp8e4 DoubleRow matmuls for the q/k projections (real k-tile pairing)
    and for S^T (stride-0 self-pairing; the x2 is folded into the exp scale).
  - AV in "orientation A": O[i-tile, 65] per head accumulated over j-tiles
    (out free = 65 incl. a ones-column that yields softmax denominators).
  - exp(S) on ACT (bf16 E tiles); copies split across DVE; elementwise
    SBUF work (scales, normalize) on the otherwise-idle Pool engine.
  - Token 1024 (j tail) via per-dt linearized E row (el = 1 + g*logit);
    row i=0 (CLS query) recomputed at the end from the patched-prior row.
  - Out-projection from O^T (PE transposes); b_out added via a K=1 matmul.
"""

import numpy as np

import concourse.bass as bass
import concourse.mybir as mybir
import bass_rust as _bass_rust
from concourse.tile import TileContext
from concourse.bass_utils import run_bass_kernel_spmd

P = 128
N = 1025
D = 768
H = 12
HD = 64
KT = 6
NT = 8
SCALE = float(D) ** -0.5     # reference scale d**-0.5
GAM_S = SCALE / 2.0          # for stride-0-doubled DR paths
GAM_C = SCALE                # for real-paired DR paths
W8S = 16.0                   # fp8 weight pre-scale (avoids subnormals)
EPS = 1e-5
F32 = mybir.dt.float32
BF16 = mybir.dt.bfloat16
FP8 = mybir.dt.float8e4
AF = mybir.ActivationFunctionType
ALU = mybir.AluOpType
DRM = mybir.MatmulPerfMode.DoubleRow

NP = 1026  # even row pitch for fp8 tiles (2B-aligned slices)
CH2 = [(0, 512), (512, 512)]
CH3 = CH2 + [(1024, 1)]


def bc2(ap, k, n):
    """[k, n] AP -> [k, 2, n] stride-0 self-pair for DoubleRow."""
    return ap[:, None, :].to_broadcast((k, 2, n))


def build_core_program():
    nc = bass.Bass()

    x_d = nc.dram_tensor("x", [N, D], F32, kind="ExternalInput")
    canny_d = nc.dram_tensor("canny", [1, 32, 32], F32, kind="ExternalInput")
    noise_d = nc.dram_tensor("noise", [32, 32], F32, kind="ExternalInput")
    lnw_d = nc.dram_tensor("ln_w", [D], F32, kind="ExternalInput")
    lnb_d = nc.dram_tensor("ln_b", [D], F32, kind="ExternalInput")
    wqkv_d = nc.dram_tensor("w_qkv", [D, 3 * D], F32, kind="ExternalInput")
    wout_d = nc.dram_tensor("w_out", [D, D], F32, kind="ExternalInput")
    bout_d = nc.dram_tensor("b_out", [D], F32, kind="ExternalInput")
    out_d = nc.dram_tensor("out", [N, D], F32, kind="ExternalOutput")

    with TileContext(nc) as tc:
        with (
            tc.tile_pool(name="persist", bufs=1) as pp,
            tc.tile_pool(name="once", bufs=1) as op,
            tc.tile_pool(name="xt", bufs=2) as xp,
            tc.tile_pool(name="wt", bufs=2) as wp,
            tc.tile_pool(name="et", bufs=2) as ep,
            tc.tile_pool(name="ot", bufs=2) as otp,
            tc.tile_pool(name="dram", bufs=1, space="DRAM") as dp,
            tc.tile_pool(name="ps_s", bufs=2, space="PSUM") as ps_s,
            tc.tile_pool(name="ps_av", bufs=3, space="PSUM") as ps_av,
            tc.tile_pool(name="ps_w", bufs=1, space="PSUM") as ps_w,
        ):
            # ---------------- persistent tiles ----------------
            xnT = pp.tile([P, KT, N], BF16, name="xnT")
            xn8 = pp.tile([P, KT, NP], FP8, name="xn8")
            qT8 = pp.tile([P, KT, NP], FP8, name="qT8")
            kT8 = pp.tile([P, KT, NP], FP8, name="kT8")
            vsb = pp.tile([P, NT + 1, H, HD + 1], BF16, name="vsb")
            Osb = pp.tile([P, NT + 1, H, HD + 1], BF16, name="Osb")
            sgat = pp.tile([P, H], F32, name="sgat")
            wvall = pp.tile([P, KT, D], BF16, name="wvall")
            wout_sb = pp.tile([P, KT, D], BF16, name="wout_sb")
            el_sb = pp.tile([HD, KT, N], BF16, name="el_sb")
            e1024 = pp.tile([P, NT, H], BF16, name="e1024")
            vdup = pp.tile([33, H, HD + 1], BF16, name="vdup")
            expUc = pp.tile([P, NT + 1, H], BF16, name="expUc")
            srecip = pp.tile([P, NT + 1, H], F32, name="srecip")
            lnw_col = pp.tile([P, KT], F32, name="lnw_col")
            lnb_col = pp.tile([P, KT], F32, name="lnb_col")
            lnw16 = pp.tile([P, KT], F32, name="lnw16")
            id128 = pp.tile([P, P], BF16, name="id128")
            id12 = pp.tile([H, H], BF16, name="id12")
            ones1 = pp.tile([1, P], BF16, name="ones1")
            bb_row = pp.tile([1, D], BF16, name="bb_row")
            eps_col = pp.tile([P, 1], F32, name="eps_col")
            q0b = pp.tile([P, KT, 16], FP8, name="q0b")
            k1024b = pp.tile([P, KT, 48], FP8, name="k1024b")
            q1024b = pp.tile([P, KT, 2], FP8, name="q1024b")
            clsrow = pp.tile([H, N], F32, name="clsrow")
            e1row = pp.tile([H, N - 1], F32, name="e1row")
            expu = pp.tile([H, N], BF16, name="expu")
            sum1 = pp.tile([H, 1], F32, name="sum1")
            recip1 = pp.tile([H, 1], F32, name="recip1")
            cnrep = pp.tile([H, N - 1], BF16, name="cnrep")

            scr_cn = dp.tile([1, N - 1], F32, name="scr_cn")

            # ---------------- constants ----------------
            from concourse.masks import make_identity
            nc.vector.memset(id128[:], 0.0)
            make_identity(nc, id128[:], nomemset=True)
            nc.vector.memset(id12[:], 0.0)
            make_identity(nc, id12[:], nomemset=True)
            nc.vector.memset(ones1[:], 1.0)
            nc.vector.memset(eps_col[:], EPS)
            nc.vector.memset(vsb[:, :, :, HD : HD + 1], 1.0)
            nc.vector.memset(q0b[:], 0.0)
            nc.vector.memset(k1024b[:], 0.0)
            nc.vector.memset(q1024b[:], 0.0)
            warm = op.tile([1, 1], F32, name="warm")
            nc.scalar.activation(warm[:], eps_col[0:1, :], AF.Exp)
            nc.sync.dma_start(
                lnw_col[:], lnw_d[:].rearrange("(k p) -> p k", p=P)
            )
            nc.sync.dma_start(
                lnb_col[:], lnb_d[:].rearrange("(k p) -> p k", p=P)
            )
            bbf = op.tile([1, D], F32, name="bbf")
            nc.sync.dma_start(bbf[:], bout_d[None, :])
            nc.gpsimd.tensor_copy(bb_row[:], bbf[:])
            nc.gpsimd.tensor_scalar(lnw16[:], lnw_col[:], W8S, None, ALU.mult)
            # PE warm-up spin to open the clock gate
            for _w in range(24):
                pwarm = ps_w.tile([P, P], F32, name="pwarm", tag="pw")
                nc.tensor.matmul(pwarm[:], id128[:], id128[:], start=True, stop=True)

            # ---------------- A: LayerNorm + transpose ----------------
            for tt in range(NT + 1):
                rows = P if tt < NT else 1
                xt = xp.tile([P, D], F32, name="xt", tag="xt")
                nc.sync.dma_start(xt[:rows], x_d[tt * P : tt * P + rows, :])
                stats = xp.tile([P, 2, 6], F32, name="stats", tag="st")
                mv = xp.tile([P, 2], F32, name="mv", tag="mv")
                nc.vector.bn_stats(stats[:rows, 0, :], xt[:rows, 0 : D // 2])
                nc.vector.bn_stats(stats[:rows, 1, :], xt[:rows, D // 2 : D])
                nc.vector.bn_aggr(mv[:rows], stats[:rows])
                lnv = xp.tile([P, 1], F32, name="lnv", tag="lnv")
                rstd = xp.tile([P, 1], F32, name="rstd", tag="rstd")
                nc.scalar.activation(
                    lnv[:rows], mv[:rows, 1:2], AF.Ln, bias=eps_col[:rows, 0:1]
                )
                nc.scalar.activation(rstd[:rows], lnv[:rows], AF.Exp, scale=-0.5)
                xc = xp.tile([P, D], BF16, name="xc", tag="xc")
                nc.gpsimd.tensor_scalar(
                    xc[:rows],
                    xt[:rows],
                    mv[:rows, 0:1],
                    rstd[:rows, 0:1],
                    ALU.subtract,
                    ALU.mult,
                )
                pst = ps_s.tile([P, KT, P], BF16, name="pst", tag="ss")
                for kt in range(KT):
                    nc.tensor.matmul(
                        pst[:, kt, :rows],
                        xc[:rows, kt * P : (kt + 1) * P],
                        id128[:rows, :rows],
                        is_transpose=True,
                        start=(kt == 0),
                        stop=(kt == KT - 1),
                    )
                nc.vector.tensor_copy(
                    xnT[:, :, tt * P : tt * P + rows], pst[:, :, :rows]
                )
                for kt in range(KT):
                    nc.gpsimd.tensor_scalar(
                        xnT[:, kt, tt * P : tt * P + rows],
                        xnT[:, kt, tt * P : tt * P + rows],
                        lnb_col[:, kt : kt + 1],
                        None, ALU.add,
                    )
                nc.gpsimd.tensor_copy(
                    xn8[:, :, tt * P : tt * P + rows],
                    xnT[:, :, tt * P : tt * P + rows],
                )

            # ---------------- priors (independent) ----------------
            crow = op.tile([1, N - 1], F32, name="crow")
            nrow = op.tile([1, N - 1], F32, name="nrow")
            csum = op.tile([1, 1], F32, name="csum")
            nsum = op.tile([1, 1], F32, name="nsum")
            crcp = op.tile([1, 1], F32, name="crcp")
            nrcp = op.tile([1, 1], F32, name="nrcp")
            nc.sync.dma_start(crow[:], canny_d[:].rearrange("a b c -> a (b c)"))
            nc.sync.dma_start(nrow[:], noise_d[:].rearrange("b c -> (b c)")[None, :])
            nc.scalar.activation(crow[:], crow[:], AF.Identity, accum_out=csum[:])
            nc.scalar.activation(nrow[:], nrow[:], AF.Identity, accum_out=nsum[:])
            nc.vector.tensor_scalar_add(csum[:], csum[:], float(N - 1))
            nc.vector.reciprocal(crcp[:], csum[:])
            nc.vector.reciprocal(nrcp[:], nsum[:])
            nc.vector.tensor_scalar(
                crow[:], crow[:], 1.0, crcp[:, 0:1], ALU.add, ALU.mult
            )
            nc.vector.tensor_scalar_mul(nrow[:], nrow[:], nrcp[:, 0:1])
            nc.vector.tensor_add(crow[:], crow[:], nrow[:])
            nc.sync.dma_start(scr_cn[:], crow[:])
            nc.gpsimd.dma_start(cnrep[:], scr_cn[:].to_broadcast((H, N - 1)))

            # ---------------- weights: w_v (w_out deferred to dt loop) ----
            for kt in range(KT):
                wvf = wp.tile([P, D], F32, name="wvf", tag="wvf")
                nc.scalar.dma_start(
                    wvf[:], wqkv_d[kt * P : (kt + 1) * P, 2 * D : 3 * D]
                )
                nc.gpsimd.tensor_scalar(
                    wvall[:, kt, :], wvf[:], lnw_col[:, kt : kt + 1],
                    None, ALU.mult,
                )

            def load_wout(kt):
                wof = wp.tile([P, D], F32, name="wof", tag="wof")
                nc.scalar.dma_start(wof[:], wout_d[kt * P : (kt + 1) * P, :])
                nc.gpsimd.tensor_copy(wout_sb[:, kt, :], wof[:])

            def vproj_tt(tt):
                rows = P if tt < NT else 1
                for c2 in range(2):
                    pb = ps_av.tile([P, 512], F32, name="pb", tag="av")
                    for kt in range(KT):
                        nc.tensor.matmul(
                            pb[:rows, :384],
                            xnT[:, kt, tt * P : tt * P + rows],
                            wvall[:, kt, c2 * 384 : (c2 + 1) * 384],
                            start=(kt == 0),
                            stop=(kt == KT - 1),
                        )
                    nc.vector.tensor_copy(
                        vsb[:rows, tt, 6 * c2 : 6 * c2 + 6, 0:HD],
                        pb[:rows, :384].rearrange("p (h f) -> p h f", h=6),
                    )

            def vdup_fill():
                # duplicate the token-1024 v row at partitions 0 and 32
                nc.gpsimd.tensor_copy(vdup[0:1, :, :], vsb[0:1, NT, :, :])
                nc.gpsimd.tensor_copy(vdup[32:33, :, :], vsb[0:1, NT, :, :])

            # ---------------- q/k projections (fp8 DR) ----------------
            def project_mt(mt):
                """mt 0..5 -> q d-tile mt; mt 6..11 -> k d-tile mt-6."""
                dt_ = mt % KT
                dst = qT8 if mt < KT else kT8
                wtile = wp.tile([P, KT, P], F32, name="wtile", tag="wtile")
                nc.sync.dma_start(
                    wtile[:],
                    wqkv_d[:, mt * P : (mt + 1) * P].rearrange(
                        "(k p) c -> p k c", p=P
                    ),
                )
                w8 = wp.tile([P, KT, P], FP8, name="w8", tag="w8")
                for kt in range(KT):
                    nc.gpsimd.tensor_scalar(
                        w8[:, kt, :], wtile[:, kt, :], lnw16[:, kt : kt + 1],
                        None, ALU.mult,
                    )
                for cs, cl in CH3:
                    pq = ps_av.tile([P, 512], F32, name="pq", tag="av")
                    for pr in range(3):
                        nc.tensor.matmul(
                            pq[:, :cl],
                            w8[:, 2 * pr : 2 * pr + 2, :],
                            xn8[:, 2 * pr : 2 * pr + 2, cs : cs + cl],
                            start=(pr == 0),
                            stop=(pr == 2),
                            perf_mode=DRM,
                        )
                    nc.vector.tensor_scalar(
                        dst[:, dt_, cs : cs + cl], pq[:, :cl], 1.0 / W8S,
                        None, ALU.mult,
                    )

            project_mt(0)
            project_mt(6)

            # ---------------- attention per d-tile (2 heads) ----------------
            for dt_ in range(KT):
                h0, h1 = 2 * dt_, 2 * dt_ + 1
                # install block-diag columns for this dt
                for par, h in ((0, h0), (1, h1)):
                    qb = par * HD
                    nc.vector.tensor_copy(
                        q0b[qb : qb + HD, dt_, h : h + 1],
                        qT8[qb : qb + HD, dt_, 0:1],
                    )
                    nc.vector.tensor_copy(
                        k1024b[qb : qb + HD, dt_, 32 * par : 32 * par + 1],
                        kT8[qb : qb + HD, dt_, 1024:1025],
                    )
                    nc.vector.tensor_copy(
                        q1024b[qb : qb + HD, dt_, par : par + 1],
                        qT8[qb : qb + HD, dt_, 1024:1025],
                    )
                # el row (j=1024) for both heads, linearized exp
                pel = ps_s.tile([48, 1024], F32, name="pel", tag="ss")
                for cs, cl in CH2:
                    nc.tensor.matmul(
                        pel[:, cs : cs + cl],
                        bc2(k1024b[:, dt_, :], P, 48),
                        bc2(qT8[:, dt_, cs : cs + cl], P, cl),
                        start=True,
                        stop=True,
                        perf_mode=DRM,
                    )
                nc.vector.tensor_scalar(
                    el_sb[0:33, dt_, 0:1024], pel[0:33, :], GAM_S, 1.0,
                    ALU.mult, ALU.add,
                )
                pel1 = ps_w.tile([48, 16], F32, name="pel1", tag="pw")
                nc.tensor.matmul(
                    pel1[:, 0:1],
                    bc2(k1024b[:, dt_, :], P, 48),
                    bc2(qT8[:, dt_, 1024:1025], P, 1),
                    start=True,
                    stop=True,
                    perf_mode=DRM,
                )
                nc.vector.tensor_scalar(
                    el_sb[0:33, dt_, 1024:1025], pel1[0:33, 0:1], GAM_S, 1.0,
                    ALU.mult, ALU.add,
                )
                # e1024 (E[1024, j] for j<1024), both heads: one chain, one bank
                pe1024 = ps_w.tile([P, 16], F32, name="pe1024", tag="pw")
                pe1024v = pe1024[:].rearrange("p (a b) -> p a b", a=8)
                for jt in range(NT):
                    nc.tensor.matmul(
                        pe1024v[:, jt, :],
                        bc2(kT8[:, dt_, jt * P : (jt + 1) * P], P, P),
                        bc2(q1024b[:, dt_, :], P, 2),
                        start=(jt == 0),
                        stop=(jt == NT - 1),
                        perf_mode=DRM,
                    )
                nc.scalar.activation(
                    e1024[:, :, h0 : h0 + 2], pe1024v[:], AF.Exp, scale=GAM_S
                )
                # S^T + exp into a full per-head E tile
                Ets = {}
                for par, h in ((0, h0), (1, h1)):
                    qb = par * HD
                    Etf = ep.tile([P, NT, 1024], BF16, name="Etf", tag="Et")
                    Ets[h] = Etf
                    for jt in range(NT):
                        psS = ps_s.tile([P, 1024], F32, name="psS", tag="ss")
                        for cs, cl in CH2:
                            nc.tensor.matmul(
                                psS[:, cs : cs + cl],
                                bc2(kT8[qb : qb + HD, dt_, jt * P : (jt + 1) * P], HD, P),
                                bc2(qT8[qb : qb + HD, dt_, cs : cs + cl], HD, cl),
                                start=True,
                                stop=True,
                                perf_mode=DRM,
                            )
                        nc.scalar.activation(
                            Etf[:, jt, :], psS[:], AF.Exp, scale=GAM_S
                        )
                        # interleave next-dt projections into the S/exp stream
                        if dt_ < KT - 1 and par == 1:
                            if jt == 2:
                                project_mt(dt_ + 1)
                            elif jt == 5:
                                project_mt(dt_ + 7)
                # AV: one accumulation chain (= one PSUM bank) per (head, i-tile)
                for par, h in ((0, h0), (1, h1)):
                    Etf = Ets[h]
                    for it in range(NT + 1):
                        ri = P if it < NT else 1
                        pav = ps_av.tile([P, 512], F32, name="pav", tag="av")
                        for jt in range(NT):
                            if it < NT:
                                lhsT = Etf[:, jt, it * P : (it + 1) * P]
                            else:
                                lhsT = e1024[:, jt, h : h + 1]
                            nc.tensor.matmul(
                                pav[:ri, 0 : HD + 1],
                                lhsT,
                                vsb[:, jt, h, :],
                                start=(jt == 0),
                                stop=False,
                            )
                        if it < NT:
                            tl = el_sb[32 * par : 32 * par + 1, dt_, it * P : (it + 1) * P]
                        else:
                            tl = el_sb[32 * par : 32 * par + 1, dt_, 1024:1025]
                        nc.tensor.matmul(
                            pav[:ri, 0 : HD + 1],
                            tl,
                            vdup[32 * par : 32 * par + 1, h, :],
                            start=False,
                            stop=True,
                        )
                        nc.vector.tensor_copy(
                            Osb[:ri, it, h, :], pav[:ri, 0 : HD + 1]
                        )

            # ---------------- CLS row (i=0) ----------------
            for cs, cl in CH3:
                pc = ps_w.tile([16, 512], F32, name="pc", tag="pw")
                for pr in range(3):
                    nc.tensor.matmul(
                        pc[:, :cl],
                        q0b[:, 2 * pr : 2 * pr + 2, :],
                        kT8[:, 2 * pr : 2 * pr + 2, cs : cs + cl],
                        start=(pr == 0),
                        stop=(pr == 2),
                        perf_mode=DRM,
                    )
                nc.vector.tensor_scalar(
                    clsrow[:, cs : cs + cl], pc[0:H, :cl], GAM_C, None, ALU.mult
                )
            # first softmax over patch keys + priors
            nc.scalar.activation(e1row[:], clsrow[:, 1:N], AF.Exp, accum_out=sum1[:])
            nc.vector.reciprocal(recip1[:], sum1[:])
            nc.vector.tensor_scalar_mul(e1row[:], e1row[:], recip1[:, 0:1])
            nc.vector.tensor_tensor(
                clsrow[:, 1:N], e1row[:], cnrep[:], ALU.add
            )
            nc.scalar.activation(expu[:], clsrow[:], AF.Exp)
            for it in range(NT + 1):
                rows = P if it < NT else 1
                pu = ps_w.tile([P, H], BF16, name="pu", tag="pw")
                nc.tensor.transpose(
                    pu[:rows, :],
                    expu[:, it * P : it * P + rows],
                    id12[:],
                )
                nc.vector.tensor_copy(expUc[:rows, it, :], pu[:rows, :])

            # row 0 of O per head, from the corrected CLS attention row
            for h in range(H):
                pr0 = ps_w.tile([1, 512], F32, name="pr0", tag="pw")
                for jt in range(NT + 1):
                    rows = P if jt < NT else 1
                    nc.tensor.matmul(
                        pr0[0:1, 0 : HD + 1],
                        expUc[:rows, jt, h : h + 1],
                        vsb[:rows, jt, h, :],
                        start=(jt == 0),
                        stop=(jt == NT),
                    )
                nc.vector.tensor_copy(Osb[0:1, 0, h, :], pr0[0:1, 0 : HD + 1])

            # ---------------- normalize + O^T + out projection ----------
            for it in range(NT + 1):
                rows = P if it < NT else 1
                nc.vector.tensor_copy(sgat[:rows], Osb[:rows, it, :, HD : HD + 1])
                nc.vector.reciprocal(srecip[:rows, it, :], sgat[:rows])
                for h in range(H):
                    nc.gpsimd.tensor_scalar(
                        Osb[:rows, it, h, 0:HD],
                        Osb[:rows, it, h, 0:HD],
                        srecip[:rows, it, h : h + 1],
                        None,
                        ALU.mult,
                    )
                psT = ps_s.tile([P, 2, 1024], BF16, name="psT", tag="ss")
                for h in range(H):
                    qb2 = (h % 2) * HD
                    nc.tensor.matmul(
                        psT[qb2 : qb2 + HD, h % 2, (h // 2) * P : (h // 2) * P + rows],
                        Osb[:rows, it, h, 0:HD],
                        id128[:rows, :rows],
                        is_transpose=True,
                        start=(h < 2),
                        stop=(h >= H - 2),
                    )
                otb = otp.tile([P, KT, P], BF16, name="otb", tag="otb")
                for par2 in range(2):
                    qb2 = par2 * HD
                    nc.vector.tensor_copy(
                        otb[qb2 : qb2 + HD, :, :rows],
                        psT[qb2 : qb2 + HD, par2, 0 : KT * P].rearrange(
                            "p (k f) -> p k f", k=KT
                        )[:, :, :rows],
                    )
                psO = ps_s.tile([P, 2, 512], F32, name="psO", tag="ss")
                for c2 in range(2):
                    for kt in range(KT):
                        nc.tensor.matmul(
                            psO[:rows, c2, 0:384],
                            otb[:, kt, :rows],
                            wout_sb[:, kt, c2 * 384 : (c2 + 1) * 384],
                            start=(kt == 0),
                            stop=False,
                        )
                    nc.tensor.matmul(
                        psO[:rows, c2, 0:384],
                        ones1[0:1, :rows],
                        bb_row[0:1, c2 * 384 : (c2 + 1) * 384],
                        start=False,
                        stop=True,
                    )
                osb_out = otp.tile([P, D], F32, name="osb_out", tag="oout")
                nc.vector.tensor_copy(
                    osb_out[:rows].rearrange("p (a b) -> p a b", a=2),
                    psO[:rows, :, 0:384],
                )
                nc.sync.dma_start(
                    out_d[it * P : it * P + rows, :], osb_out[:rows]
                )

    _bass_rust.generate_event_semaphores(nc)
    return nc


_NC_CACHE = None


def kernel(**inputs) -> np.ndarray:
    global _NC_CACHE
    x = np.ascontiguousarray(np.asarray(inputs["x"], dtype=np.float32))
    canny = np.ascontiguousarray(np.asarray(inputs["canny"], dtype=np.float32))
    noise = np.ascontiguousarray(np.asarray(inputs["noise"], dtype=np.float32))
    ln_w = np.ascontiguousarray(np.asarray(inputs["ln_w"], dtype=np.float32))
    ln_b = np.ascontiguousarray(np.asarray(inputs["ln_b"], dtype=np.float32))
    w_qkv = np.ascontiguousarray(np.asarray(inputs["w_qkv"], dtype=np.float32))
    w_out = np.ascontiguousarray(np.asarray(inputs["w_out"], dtype=np.float32))
    b_out = np.ascontiguousarray(np.asarray(inputs["b_out"], dtype=np.float32))

    B = x.shape[0]
    assert B == 8, f"expected batch 8, got {B}"

    if _NC_CACHE is None:
        _NC_CACHE = build_core_program()
    nc = _NC_CACHE

    in_maps = [
        {
            "x": x[b],
            "canny": canny[b],
            "noise": noise[b],
            "ln_w": ln_w,
            "ln_b": ln_b,
            "w_qkv": w_qkv,
            "w_out": w_out,
            "b_out": b_out,
        }
        for b in range(B)
    ]
    res = run_bass_kernel_spmd(nc, in_maps, core_ids=list(range(B)))
    out = np.stack([res.results[b]["out"] for b in range(B)], axis=0)
    return out.astype(np.float32)


# revision 3
# speedup vs baseline: 1.1159x; 1.1159x over previous
"""Trainium2 Bass kernel for nn_Attention_41678362640976 (v2).

ViT-style attention with CLS-row prior injection. Data-parallel over batch:
one batch element per NeuronCore (B == 8 == n_cores).

Key design (cost-model-driven):
  - fp8e4 DoubleRow matmuls for the q/k projections (real k-tile pairing)
    and for S^T (stride-0 self-pairing; the x2 is folded into the exp scale).
  - AV in "orientation A": O[i-tile, 65] per head accumulated over j-tiles
    (out free = 65 incl. a ones-column that yields softmax denominators).
  - exp(S) on ACT (bf16 E tiles); copies split across DVE; elementwise
    SBUF work (scales, normalize) on the otherwise-idle Pool engine.
  - Token 1024 (j tail) via per-dt linearized E row (el = 1 + g*logit);
    row i=0 (CLS query) recomputed at the end from the patched-prior row.
  - Out-projection from O^T (PE transposes); b_out added via a K=1 matmul.
"""

import numpy as np

import concourse.bass as bass
import concourse.mybir as mybir
import bass_rust as _bass_rust
from concourse.tile import TileContext
from concourse.bass_utils import run_bass_kernel_spmd

P = 128
N = 1025
D = 768
H = 12
HD = 64
KT = 6
NT = 8
SCALE = float(D) ** -0.5     # reference scale d**-0.5
GAM_S = SCALE / 2.0          # for stride-0-doubled DR paths
GAM_C = SCALE                # for real-paired DR paths
W8S = 16.0                   # fp8 weight pre-scale (avoids subnormals)
EPS = 1e-5
F32 = mybir.dt.float32
BF16 = mybir.dt.bfloat16
FP8 = mybir.dt.float8e4
AF = mybir.ActivationFunctionType
ALU = mybir.AluOpType
DRM = mybir.MatmulPerfMode.DoubleRow

NP = 1026  # even row pitch for fp8 tiles (2B-aligned slices)
CH2 = [(0, 512), (512, 512)]
CH3 = CH2 + [(1024, 1)]


def bc2(ap, k, n):
    """[k, n] AP -> [k, 2, n] stride-0 self-pair for DoubleRow."""
    return ap[:, None, :].to_broadcast((k, 2, n))


def build_core_program():
    nc = bass.Bass()

    x_d = nc.dram_tensor("x", [N, D], F32, kind="ExternalInput")
    canny_d = nc.dram_tensor("canny", [1, 32, 32], F32, kind="ExternalInput")
    noise_d = nc.dram_tensor("noise", [32, 32], F32, kind="ExternalInput")
    lnw_d = nc.dram_tensor("ln_w", [D], F32, kind="ExternalInput")
    lnb_d = nc.dram_tensor("ln_b", [D], F32, kind="ExternalInput")
    wqkv_d = nc.dram_tensor("w_qkv", [D, 3 * D], F32, kind="ExternalInput")
    wout_d = nc.dram_tensor("w_out", [D, D], F32, kind="ExternalInput")
    bout_d = nc.dram_tensor("b_out", [D], F32, kind="ExternalInput")
    out_d = nc.dram_tensor("out", [N, D], F32, kind="ExternalOutput")

    with TileContext(nc) as tc:
        with (
            tc.tile_pool(name="persist", bufs=1) as pp,
            tc.tile_pool(name="once", bufs=1) as op,
            tc.tile_pool(name="xt", bufs=2) as xp,
            tc.tile_pool(name="wt", bufs=2) as wp,
            tc.tile_pool(name="et", bufs=2) as ep,
            tc.tile_pool(name="ot", bufs=2) as otp,
            tc.tile_pool(name="dram", bufs=1, space="DRAM") as dp,
            tc.tile_pool(name="ps_s", bufs=2, space="PSUM") as ps_s,
            tc.tile_pool(name="ps_av", bufs=3, space="PSUM") as ps_av,
            tc.tile_pool(name="ps_w", bufs=1, space="PSUM") as ps_w,
        ):
            # ---------------- persistent tiles ----------------
            xnT = pp.tile([P, KT, N], BF16, name="xnT")
            xn8 = pp.tile([P, KT, NP], FP8, name="xn8")
            qT8 = pp.tile([P, KT, NP], FP8, name="qT8")
            kT8 = pp.tile([P, KT, NP], FP8, name="kT8")
            vsb = pp.tile([P, NT + 1, H, HD + 1], BF16, name="vsb")
            Osb = pp.tile([P, NT + 1, H, HD + 1], BF16, name="Osb")
            sgat = pp.tile([P, H], F32, name="sgat")
            wvall = pp.tile([P, KT, D], BF16, name="wvall")
            wout_sb = pp.tile([P, KT, D], BF16, name="wout_sb")
            el_sb = pp.tile([HD, KT, N], BF16, name="el_sb")
            e1024 = pp.tile([P, NT, H], BF16, name="e1024")
            vdup = pp.tile([33, H, HD + 1], BF16, name="vdup")
            expUc = pp.tile([P, NT + 1, H], BF16, name="expUc")
            srecip = pp.tile([P, NT + 1, H], F32, name="srecip")
            lnw_col = pp.tile([P, KT], F32, name="lnw_col")
            lnb_col = pp.tile([P, KT], F32, name="lnb_col")
            lnw16 = pp.tile([P, KT], F32, name="lnw16")
            id128 = pp.tile([P, P], BF16, name="id128")
            id12 = pp.tile([H, H], BF16, name="id12")
            ones1 = pp.tile([1, P], BF16, name="ones1")
            bb_row = pp.tile([1, D], BF16, name="bb_row")
            eps_col = pp.tile([P, 1], F32, name="eps_col")
            q0b = pp.tile([P, KT, 16], FP8, name="q0b")
            k1024b = pp.tile([P, KT, 48], FP8, name="k1024b")
            q1024b = pp.tile([P, KT, 2], FP8, name="q1024b")
            clsrow = pp.tile([H, N], F32, name="clsrow")
            e1row = pp.tile([H, N - 1], F32, name="e1row")
            expu = pp.tile([H, N], BF16, name="expu")
            sum1 = pp.tile([H, 1], F32, name="sum1")
            recip1 = pp.tile([H, 1], F32, name="recip1")
            cnrep = pp.tile([H, N - 1], BF16, name="cnrep")

            scr_cn = dp.tile([1, N - 1], F32, name="scr_cn")

            # ---------------- constants ----------------
            from concourse.masks import make_identity
            nc.vector.memset(id128[:], 0.0)
            make_identity(nc, id128[:], nomemset=True)
            nc.vector.memset(id12[:], 0.0)
            make_identity(nc, id12[:], nomemset=True)
            nc.vector.memset(ones1[:], 1.0)
            nc.vector.memset(eps_col[:], EPS)
            nc.vector.memset(vsb[:, :, :, HD : HD + 1], 1.0)
            nc.vector.memset(q0b[:], 0.0)
            nc.vector.memset(k1024b[:], 0.0)
            nc.vector.memset(q1024b[:], 0.0)
            warm = op.tile([1, 1], F32, name="warm")
            nc.scalar.activation(warm[:], eps_col[0:1, :], AF.Exp)
            nc.sync.dma_start(
                lnw_col[:], lnw_d[:].rearrange("(k p) -> p k", p=P)
            )
            nc.sync.dma_start(
                lnb_col[:], lnb_d[:].rearrange("(k p) -> p k", p=P)
            )
            bbf = op.tile([1, D], F32, name="bbf")
            nc.sync.dma_start(bbf[:], bout_d[None, :])
            nc.gpsimd.tensor_copy(bb_row[:], bbf[:])
            nc.gpsimd.tensor_scalar(lnw16[:], lnw_col[:], W8S, None, ALU.mult)
            # PE warm-up spin to open the clock gate
            for _w in range(24):
                pwarm = ps_w.tile([P, P], F32, name="pwarm", tag="pw")
                nc.tensor.matmul(pwarm[:], id128[:], id128[:], start=True, stop=True)

            # ---------------- A: LayerNorm + transpose ----------------
            for tt in range(NT + 1):
                rows = P if tt < NT else 1
                xt = xp.tile([P, D], F32, name="xt", tag="xt")
                nc.sync.dma_start(xt[:rows], x_d[tt * P : tt * P + rows, :])
                stats = xp.tile([P, 2, 6], F32, name="stats", tag="st")
                mv = xp.tile([P, 2], F32, name="mv", tag="mv")
                nc.vector.bn_stats(stats[:rows, 0, :], xt[:rows, 0 : D // 2])
                nc.vector.bn_stats(stats[:rows, 1, :], xt[:rows, D // 2 : D])
                nc.vector.bn_aggr(mv[:rows], stats[:rows])
                lnv = xp.tile([P, 1], F32, name="lnv", tag="lnv")
                rstd = xp.tile([P, 1], F32, name="rstd", tag="rstd")
                nc.scalar.activation(
                    lnv[:rows], mv[:rows, 1:2], AF.Ln, bias=eps_col[:rows, 0:1]
                )
                nc.scalar.activation(rstd[:rows], lnv[:rows], AF.Exp, scale=-0.5)
                xc = xp.tile([P, D], BF16, name="xc", tag="xc")
                nc.vector.tensor_scalar(
                    xc[:rows],
                    xt[:rows],
                    mv[:rows, 0:1],
                    rstd[:rows, 0:1],
                    ALU.subtract,
                    ALU.mult,
                )
                pst = ps_s.tile([P, KT, P], BF16, name="pst", tag="ss")
                for kt in range(KT):
                    nc.tensor.matmul(
                        pst[:, kt, :rows],
                        xc[:rows, kt * P : (kt + 1) * P],
                        id128[:rows, :rows],
                        is_transpose=True,
                        start=(kt == 0),
                        stop=(kt == KT - 1),
                    )
                nc.vector.tensor_copy(
                    xnT[:, :, tt * P : tt * P + rows], pst[:, :, :rows]
                )
                for kt in range(KT):
                    nc.gpsimd.tensor_scalar(
                        xnT[:, kt, tt * P : tt * P + rows],
                        xnT[:, kt, tt * P : tt * P + rows],
                        lnb_col[:, kt : kt + 1],
                        None, ALU.add,
                    )
                nc.gpsimd.tensor_copy(
                    xn8[:, :, tt * P : tt * P + rows],
                    xnT[:, :, tt * P : tt * P + rows],
                )

            # ---------------- priors (independent) ----------------
            crow = op.tile([1, N - 1], F32, name="crow")
            nrow = op.tile([1, N - 1], F32, name="nrow")
            csum = op.tile([1, 1], F32, name="csum")
            nsum = op.tile([1, 1], F32, name="nsum")
            crcp = op.tile([1, 1], F32, name="crcp")
            nrcp = op.tile([1, 1], F32, name="nrcp")
            nc.sync.dma_start(crow[:], canny_d[:].rearrange("a b c -> a (b c)"))
            nc.sync.dma_start(nrow[:], noise_d[:].rearrange("b c -> (b c)")[None, :])
            nc.scalar.activation(crow[:], crow[:], AF.Identity, accum_out=csum[:])
            nc.scalar.activation(nrow[:], nrow[:], AF.Identity, accum_out=nsum[:])
            nc.vector.tensor_scalar_add(csum[:], csum[:], float(N - 1))
            nc.vector.reciprocal(crcp[:], csum[:])
            nc.vector.reciprocal(nrcp[:], nsum[:])
            nc.vector.tensor_scalar(
                crow[:], crow[:], 1.0, crcp[:, 0:1], ALU.add, ALU.mult
            )
            nc.vector.tensor_scalar_mul(nrow[:], nrow[:], nrcp[:, 0:1])
            nc.vector.tensor_add(crow[:], crow[:], nrow[:])
            nc.sync.dma_start(scr_cn[:], crow[:])
            nc.gpsimd.dma_start(cnrep[:], scr_cn[:].to_broadcast((H, N - 1)))

            # ---------------- weights: w_v (w_out deferred to dt loop) ----
            for kt in range(KT):
                wvf = wp.tile([P, D], F32, name="wvf", tag="wvf")
                nc.scalar.dma_start(
                    wvf[:], wqkv_d[kt * P : (kt + 1) * P, 2 * D : 3 * D]
                )
                nc.gpsimd.tensor_scalar(
                    wvall[:, kt, :], wvf[:], lnw_col[:, kt : kt + 1],
                    None, ALU.mult,
                )

            def load_wout(kt):
                wof = wp.tile([P, D], F32, name="wof", tag="wof")
                nc.scalar.dma_start(wof[:], wout_d[kt * P : (kt + 1) * P, :])
                nc.gpsimd.tensor_copy(wout_sb[:, kt, :], wof[:])

            def vproj_tt(tt):
                rows = P if tt < NT else 1
                for c2 in range(2):
                    pb = ps_av.tile([P, 512], F32, name="pb", tag="av")
                    for kt in range(KT):
                        nc.tensor.matmul(
                            pb[:rows, :384],
                            xnT[:, kt, tt * P : tt * P + rows],
                            wvall[:, kt, c2 * 384 : (c2 + 1) * 384],
                            start=(kt == 0),
                            stop=(kt == KT - 1),
                        )
                    nc.vector.tensor_copy(
                        vsb[:rows, tt, 6 * c2 : 6 * c2 + 6, 0:HD],
                        pb[:rows, :384].rearrange("p (h f) -> p h f", h=6),
                    )

            def vdup_fill():
                # duplicate the token-1024 v row at partitions 0 and 32
                nc.gpsimd.tensor_copy(vdup[0:1, :, :], vsb[0:1, NT, :, :])
                nc.gpsimd.tensor_copy(vdup[32:33, :, :], vsb[0:1, NT, :, :])

            # ---------------- q/k projections (fp8 DR) ----------------
            def project_mt(mt):
                """mt 0..5 -> q d-tile mt; mt 6..11 -> k d-tile mt-6."""
                dt_ = mt % KT
                dst = qT8 if mt < KT else kT8
                wtile = wp.tile([P, KT, P], F32, name="wtile", tag="wtile")
                nc.sync.dma_start(
                    wtile[:],
                    wqkv_d[:, mt * P : (mt + 1) * P].rearrange(
                        "(k p) c -> p k c", p=P
                    ),
                )
                w8 = wp.tile([P, KT, P], FP8, name="w8", tag="w8")
                for kt in range(KT):
                    nc.gpsimd.tensor_scalar(
                        w8[:, kt, :], wtile[:, kt, :], lnw16[:, kt : kt + 1],
                        None, ALU.mult,
                    )
                for cs, cl in CH3:
                    pq = ps_av.tile([P, 512], F32, name="pq", tag="av")
                    for pr in range(3):
                        nc.tensor.matmul(
                            pq[:, :cl],
                            w8[:, 2 * pr : 2 * pr + 2, :],
                            xn8[:, 2 * pr : 2 * pr + 2, cs : cs + cl],
                            start=(pr == 0),
                            stop=(pr == 2),
                            perf_mode=DRM,
                        )
                    nc.vector.tensor_scalar(
                        dst[:, dt_, cs : cs + cl], pq[:, :cl], 1.0 / W8S,
                        None, ALU.mult,
                    )

            project_mt(0)
            project_mt(6)


            def installs(dtx):
                for par, h in ((0, 2 * dtx), (1, 2 * dtx + 1)):
                    qb = par * HD
                    nc.vector.tensor_copy(
                        q0b[qb : qb + HD, dtx, h : h + 1],
                        qT8[qb : qb + HD, dtx, 0:1],
                    )
                    nc.vector.tensor_copy(
                        k1024b[qb : qb + HD, dtx, 32 * par : 32 * par + 1],
                        kT8[qb : qb + HD, dtx, 1024:1025],
                    )
                    nc.vector.tensor_copy(
                        q1024b[qb : qb + HD, dtx, par : par + 1],
                        qT8[qb : qb + HD, dtx, 1024:1025],
                    )

            def el_chunk(dtx, ci):
                cs, cl = CH2[ci]
                pel = ps_w.tile([48, 512], F32, name="pel", tag="pw")
                nc.tensor.matmul(
                    pel[:, :cl],
                    bc2(k1024b[:, dtx, :], P, 48),
                    bc2(qT8[:, dtx, cs : cs + cl], P, cl),
                    start=True,
                    stop=True,
                    perf_mode=DRM,
                )
                nc.vector.tensor_scalar(
                    el_sb[0:33, dtx, cs : cs + cl], pel[0:33, :cl], GAM_S,
                    1.0, ALU.mult, ALU.add,
                )

            def el_corner(dtx):
                pel1 = ps_w.tile([48, 16], F32, name="pel1", tag="pw")
                nc.tensor.matmul(
                    pel1[:, 0:1],
                    bc2(k1024b[:, dtx, :], P, 48),
                    bc2(qT8[:, dtx, 1024:1025], P, 1),
                    start=True,
                    stop=True,
                    perf_mode=DRM,
                )
                nc.vector.tensor_scalar(
                    el_sb[0:33, dtx, 1024:1025], pel1[0:33, 0:1], GAM_S,
                    1.0, ALU.mult, ALU.add,
                )

            def e1024_phase(dtx):
                pe1024 = ps_w.tile([P, 16], F32, name="pe1024", tag="pw")
                pe1024v = pe1024[:].rearrange("p (a b) -> p a b", a=8)
                for jt in range(NT):
                    nc.tensor.matmul(
                        pe1024v[:, jt, :],
                        bc2(kT8[:, dtx, jt * P : (jt + 1) * P], P, P),
                        bc2(q1024b[:, dtx, :], P, 2),
                        start=(jt == 0),
                        stop=(jt == NT - 1),
                        perf_mode=DRM,
                    )
                nc.scalar.activation(
                    e1024[:, :, 2 * dtx : 2 * dtx + 2], pe1024v[:],
                    AF.Exp, scale=GAM_S,
                )

            def dt_prologue(dtx):
                installs(dtx)
                el_chunk(dtx, 0)
                el_chunk(dtx, 1)
                el_corner(dtx)
                e1024_phase(dtx)

            def av_phase(dtx, Ets_):
                hh0, hh1 = 2 * dtx, 2 * dtx + 1
                for it in range(NT + 1):
                    ri = P if it < NT else 1
                    for par, h in ((0, hh0), (1, hh1)):
                        pav = ps_av.tile([P, 512], F32, name="pav", tag="av")
                        for jt in range(NT):
                            if it < NT:
                                lhsT = Ets_[h][:, jt, it * P : (it + 1) * P]
                            else:
                                lhsT = e1024[:, jt, h : h + 1]
                            nc.tensor.matmul(
                                pav[:ri, 0 : HD + 1],
                                lhsT,
                                vsb[:, jt, h, :],
                                start=(jt == 0),
                                stop=False,
                            )
                        if it < NT:
                            tl = el_sb[32 * par : 32 * par + 1, dtx, it * P : (it + 1) * P]
                        else:
                            tl = el_sb[32 * par : 32 * par + 1, dtx, 1024:1025]
                        nc.tensor.matmul(
                            pav[:ri, 0 : HD + 1],
                            tl,
                            vdup[32 * par : 32 * par + 1, h, :],
                            start=False,
                            stop=True,
                        )
                        nc.vector.tensor_copy(
                            Osb[:ri, it, h, :], pav[:ri, 0 : HD + 1]
                        )
                    nc.vector.tensor_copy(
                        sgat[:ri, 0:2],
                        Osb[:ri, it, hh0 : hh0 + 2, HD : HD + 1],
                    )
                    nc.vector.reciprocal(
                        srecip[:ri, it, hh0 : hh0 + 2], sgat[:ri, 0:2]
                    )
                    for par, h in ((0, hh0), (1, hh1)):
                        nc.gpsimd.tensor_scalar(
                            Osb[:ri, it, h, 0:HD],
                            Osb[:ri, it, h, 0:HD],
                            srecip[:ri, it, h : h + 1],
                            None,
                            ALU.mult,
                        )

            def ot_group(dtx, g):
                hh0, hh1 = 2 * dtx, 2 * dtx + 1
                psT = ps_w.tile([P, 3, P], BF16, name="psT", tag="pw")
                for par, h in ((0, hh0), (1, hh1)):
                    qb2 = par * HD
                    for idx, it in enumerate((3 * g, 3 * g + 1, 3 * g + 2)):
                        ri = P if it < NT else 1
                        nc.tensor.matmul(
                            psT[qb2 : qb2 + HD, idx, :ri],
                            Osb[:ri, it, h, 0:HD],
                            id128[:ri, :ri],
                            is_transpose=True,
                            start=(idx == 0),
                            stop=(idx == 2),
                        )
                    nc.vector.tensor_copy(
                        otball[qb2 : qb2 + HD, dtx, 3 * g : 3 * g + 3, :],
                        psT[qb2 : qb2 + HD, :, :],
                    )

            # ---------------- attention per d-tile (2 heads) ----------------
            for dt_ in range(KT):
                h0, h1 = 2 * dt_, 2 * dt_ + 1
                # install block-diag columns for this dt
                for par, h in ((0, h0), (1, h1)):
                    qb = par * HD
                    nc.vector.tensor_copy(
                        q0b[qb : qb + HD, dt_, h : h + 1],
                        qT8[qb : qb + HD, dt_, 0:1],
                    )
                    nc.vector.tensor_copy(
                        k1024b[qb : qb + HD, dt_, 32 * par : 32 * par + 1],
                        kT8[qb : qb + HD, dt_, 1024:1025],
                    )
                    nc.vector.tensor_copy(
                        q1024b[qb : qb + HD, dt_, par : par + 1],
                        qT8[qb : qb + HD, dt_, 1024:1025],
                    )
                # el row (j=1024) for both heads, linearized exp
                pel = ps_s.tile([48, 1024], F32, name="pel", tag="ss")
                for cs, cl in CH2:
                    nc.tensor.matmul(
                        pel[:, cs : cs + cl],
                        bc2(k1024b[:, dt_, :], P, 48),
                        bc2(qT8[:, dt_, cs : cs + cl], P, cl),
                        start=True,
                        stop=True,
                        perf_mode=DRM,
                    )
                nc.vector.tensor_scalar(
                    el_sb[0:33, dt_, 0:1024], pel[0:33, :], GAM_S, 1.0,
                    ALU.mult, ALU.add,
                )
                pel1 = ps_w.tile([48, 16], F32, name="pel1", tag="pw")
                nc.tensor.matmul(
                    pel1[:, 0:1],
                    bc2(k1024b[:, dt_, :], P, 48),
                    bc2(qT8[:, dt_, 1024:1025], P, 1),
                    start=True,
                    stop=True,
                    perf_mode=DRM,
                )
                nc.vector.tensor_scalar(
                    el_sb[0:33, dt_, 1024:1025], pel1[0:33, 0:1], GAM_S, 1.0,
                    ALU.mult, ALU.add,
                )
                # e1024 (E[1024, j] for j<1024), both heads: one chain, one bank
                pe1024 = ps_w.tile([P, 16], F32, name="pe1024", tag="pw")
                pe1024v = pe1024[:].rearrange("p (a b) -> p a b", a=8)
                for jt in range(NT):
                    nc.tensor.matmul(
                        pe1024v[:, jt, :],
                        bc2(kT8[:, dt_, jt * P : (jt + 1) * P], P, P),
                        bc2(q1024b[:, dt_, :], P, 2),
                        start=(jt == 0),
                        stop=(jt == NT - 1),
                        perf_mode=DRM,
                    )
                nc.scalar.activation(
                    e1024[:, :, h0 : h0 + 2], pe1024v[:], AF.Exp, scale=GAM_S
                )
                # S^T + exp into a full per-head E tile
                Ets = {}
                for par, h in ((0, h0), (1, h1)):
                    qb = par * HD
                    Etf = ep.tile([P, NT, 1024], BF16, name="Etf", tag="Et")
                    Ets[h] = Etf
                    for jt in range(NT):
                        psS = ps_s.tile([P, 1024], F32, name="psS", tag="ss")
                        for cs, cl in CH2:
                            nc.tensor.matmul(
                                psS[:, cs : cs + cl],
                                bc2(kT8[qb : qb + HD, dt_, jt * P : (jt + 1) * P], HD, P),
                                bc2(qT8[qb : qb + HD, dt_, cs : cs + cl], HD, cl),
                                start=True,
                                stop=True,
                                perf_mode=DRM,
                            )
                        nc.scalar.activation(
                            Etf[:, jt, :], psS[:], AF.Exp, scale=GAM_S
                        )
                        # interleave next-dt projections into the S/exp stream
                        if dt_ < KT - 1 and par == 1:
                            if jt == 2:
                                project_mt(dt_ + 1)
                            elif jt == 5:
                                project_mt(dt_ + 7)
                # AV: one accumulation chain (= one PSUM bank) per (head, i-tile)
                for par, h in ((0, h0), (1, h1)):
                    Etf = Ets[h]
                    for it in range(NT + 1):
                        ri = P if it < NT else 1
                        pav = ps_av.tile([P, 512], F32, name="pav", tag="av")
                        for jt in range(NT):
                            if it < NT:
                                lhsT = Etf[:, jt, it * P : (it + 1) * P]
                            else:
                                lhsT = e1024[:, jt, h : h + 1]
                            nc.tensor.matmul(
                                pav[:ri, 0 : HD + 1],
                                lhsT,
                                vsb[:, jt, h, :],
                                start=(jt == 0),
                                stop=False,
                            )
                        if it < NT:
                            tl = el_sb[32 * par : 32 * par + 1, dt_, it * P : (it + 1) * P]
                        else:
                            tl = el_sb[32 * par : 32 * par + 1, dt_, 1024:1025]
                        nc.tensor.matmul(
                            pav[:ri, 0 : HD + 1],
                            tl,
                            vdup[32 * par : 32 * par + 1, h, :],
                            start=False,
                            stop=True,
                        )
                        nc.vector.tensor_copy(
                            Osb[:ri, it, h, :], pav[:ri, 0 : HD + 1]
                        )

            # ---------------- CLS row (i=0) ----------------
            for cs, cl in CH3:
                pc = ps_w.tile([16, 512], F32, name="pc", tag="pw")
                for pr in range(3):
                    nc.tensor.matmul(
                        pc[:, :cl],
                        q0b[:, 2 * pr : 2 * pr + 2, :],
                        kT8[:, 2 * pr : 2 * pr + 2, cs : cs + cl],
                        start=(pr == 0),
                        stop=(pr == 2),
                        perf_mode=DRM,
                    )
                nc.vector.tensor_scalar(
                    clsrow[:, cs : cs + cl], pc[0:H, :cl], GAM_C, None, ALU.mult
                )
            # first softmax over patch keys + priors
            nc.scalar.activation(e1row[:], clsrow[:, 1:N], AF.Exp, accum_out=sum1[:])
            nc.vector.reciprocal(recip1[:], sum1[:])
            nc.vector.tensor_scalar_mul(e1row[:], e1row[:], recip1[:, 0:1])
            nc.vector.tensor_tensor(
                clsrow[:, 1:N], e1row[:], cnrep[:], ALU.add
            )
            nc.scalar.activation(expu[:], clsrow[:], AF.Exp)
            for it in range(NT + 1):
                rows = P if it < NT else 1
                pu = ps_w.tile([P, H], BF16, name="pu", tag="pw")
                nc.tensor.transpose(
                    pu[:rows, :],
                    expu[:, it * P : it * P + rows],
                    id12[:],
                )
                nc.vector.tensor_copy(expUc[:rows, it, :], pu[:rows, :])

            # row 0 of O per head, from the corrected CLS attention row
            for h in range(H):
                pr0 = ps_w.tile([1, 512], F32, name="pr0", tag="pw")
                for jt in range(NT + 1):
                    rows = P if jt < NT else 1
                    nc.tensor.matmul(
                        pr0[0:1, 0 : HD + 1],
                        expUc[:rows, jt, h : h + 1],
                        vsb[:rows, jt, h, :],
                        start=(jt == 0),
                        stop=(jt == NT),
                    )
                nc.vector.tensor_copy(Osb[0:1, 0, h, :], pr0[0:1, 0 : HD + 1])

            # ---------------- normalize + O^T + out projection ----------
            for it in range(NT + 1):
                rows = P if it < NT else 1
                nc.vector.tensor_copy(sgat[:rows], Osb[:rows, it, :, HD : HD + 1])
                nc.vector.reciprocal(srecip[:rows, it, :], sgat[:rows])
                for h in range(H):
                    nc.gpsimd.tensor_scalar(
                        Osb[:rows, it, h, 0:HD],
                        Osb[:rows, it, h, 0:HD],
                        srecip[:rows, it, h : h + 1],
                        None,
                        ALU.mult,
                    )
                psT = ps_s.tile([P, 2, 1024], BF16, name="psT", tag="ss")
                for h in range(H):
                    qb2 = (h % 2) * HD
                    nc.tensor.matmul(
                        psT[qb2 : qb2 + HD, h % 2, (h // 2) * P : (h // 2) * P + rows],
                        Osb[:rows, it, h, 0:HD],
                        id128[:rows, :rows],
                        is_transpose=True,
                        start=(h < 2),
                        stop=(h >= H - 2),
                    )
                otb = otp.tile([P, KT, P], BF16, name="otb", tag="otb")
                for par2 in range(2):
                    qb2 = par2 * HD
                    nc.vector.tensor_copy(
                        otb[qb2 : qb2 + HD, :, :rows],
                        psT[qb2 : qb2 + HD, par2, 0 : KT * P].rearrange(
                            "p (k f) -> p k f", k=KT
                        )[:, :, :rows],
                    )
                psO = ps_s.tile([P, 2, 512], F32, name="psO", tag="ss")
                for c2 in range(2):
                    for kt in range(KT):
                        nc.tensor.matmul(
                            psO[:rows, c2, 0:384],
                            otb[:, kt, :rows],
                            wout_sb[:, kt, c2 * 384 : (c2 + 1) * 384],
                            start=(kt == 0),
                            stop=False,
                        )
                    nc.tensor.matmul(
                        psO[:rows, c2, 0:384],
                        ones1[0:1, :rows],
                        bb_row[0:1, c2 * 384 : (c2 + 1) * 384],
                        start=False,
                        stop=True,
                    )
                osb_out = otp.tile([P, D], F32, name="osb_out", tag="oout")
                nc.vector.tensor_copy(
                    osb_out[:rows].rearrange("p (a b) -> p a b", a=2),
                    psO[:rows, :, 0:384],
                )
                nc.sync.dma_start(
                    out_d[it * P : it * P + rows, :], osb_out[:rows]
                )

    _bass_rust.generate_event_semaphores(nc)
    return nc


_NC_CACHE = None


def kernel(**inputs) -> np.ndarray:
    global _NC_CACHE
    x = np.ascontiguousarray(np.asarray(inputs["x"], dtype=np.float32))
    canny = np.ascontiguousarray(np.asarray(inputs["canny"], dtype=np.float32))
    noise = np.ascontiguousarray(np.asarray(inputs["noise"], dtype=np.float32))
    ln_w = np.ascontiguousarray(np.asarray(inputs["ln_w"], dtype=np.float32))
    ln_b = np.ascontiguousarray(np.asarray(inputs["ln_b"], dtype=np.float32))
    w_qkv = np.ascontiguousarray(np.asarray(inputs["w_qkv"], dtype=np.float32))
    w_out = np.ascontiguousarray(np.asarray(inputs["w_out"], dtype=np.float32))
    b_out = np.ascontiguousarray(np.asarray(inputs["b_out"], dtype=np.float32))

    B = x.shape[0]
    assert B == 8, f"expected batch 8, got {B}"

    if _NC_CACHE is None:
        _NC_CACHE = build_core_program()
    nc = _NC_CACHE

    in_maps = [
        {
            "x": x[b],
            "canny": canny[b],
            "noise": noise[b],
            "ln_w": ln_w,
            "ln_b": ln_b,
            "w_qkv": w_qkv,
            "w_out": w_out,
            "b_out": b_out,
        }
        for b in range(B)
    ]
    res = run_bass_kernel_spmd(nc, in_maps, core_ids=list(range(B)))
    out = np.stack([res.results[b]["out"] for b in range(B)], axis=0)
    return out.astype(np.float32)
